# revision 3
# baseline (speedup 1.0000x reference)
"""CrossNeighborAttention Trainium2 kernel (8 NeuronCores, SPMD data-parallel).

Sharding: 16 (b,t) frames over 8 cores -> 2 frames/core. Neighbor-frame K/V
("halo") is handled host-side: each core receives the img of 4 frame-slots
ordered [lo, own1, own0, hi], so frame f's keys are the contiguous slot pair
[f*1152 : f*1152+1152] in (prev|next) order and the device program is
identical on every core (pure SPMD, no collectives).

Per core:
  - xT is host-pretransposed to [C, tokens]; Q/K projections produce
    [c_out, token] tiles directly (f32r matmuls, full PE rate at N>=256).
  - RoPE: k*cos + P@(sin_swapped*k); the pair rotation P is a 128x128 matmul,
    sin_swapped is host-precomputed.
  - V is projected in [token, c_out] layout with a ones-column appended per
    head (65-wide groups) so the AV matmul emits the softmax denominator as
    output row 64 for free.
  - scores S^T[key, q] accumulate in PSUM; exp runs on ACT with the 1/8
    softmax scale fused; A^T stored bf16; AV in bf16.
  - 1/denom = exp(-ln(denom)) on ACT (DVE reciprocal is too slow, ACT
    Reciprocal is banned); broadcast across the 64 head dims via a K=1
    ones-matmul; applied by one DVE multiply.
  - output projection accumulates over head-pair chunks; bias is added
    during PSUM evacuation (fused with the copy).
"""

import sys

for p in ("/opt/trn_rl_repo", "/opt/trn_rl_repo/concourse"):
    if p not in sys.path:
        sys.path.insert(0, p)

import numpy as np

import concourse.bass as bass
import concourse.mybir as mybir
import concourse.tile as tile
from concourse import bacc
from concourse.bass_utils import run_bass_kernel_spmd

F32 = mybir.dt.float32
F32R = mybir.dt.float32r
BF16 = mybir.dt.bfloat16
EXP = mybir.ActivationFunctionType.Exp
LN = mybir.ActivationFunctionType.Ln

B, T, N, C, H = 2, 8, 576, 1024, 16
HD = C // H           # 64
NCORES = 8
SLN = N               # 576 tokens per frame-slot
NKV = 2 * N           # 1152 kv tokens per frame
CH = 288              # token chunk (>=256 keeps f32r matmuls at full rate)
KK = NKV // 128       # 9 key tiles per frame
NCI = C // 128        # 8 contraction chunks
NHP = H // 2          # 8 head-pairs (c_out chunks of 128)


def _patch_act_tables():
    """Make natural_log_exp_and_others the only set containing Exp and Ln so
    the table-load pass doesn't thrash between exp-only and ln-only sets
    (each reload costs ~2.7us on ACT). Mutates the functools.cache'd dict
    in place; set order/indices are preserved for walrus remapping."""
    import concourse.hw_specs as hw_specs

    t = hw_specs.get_activation_tables("gen3")
    for name, fns in t.items():
        if name != "natural_log_exp_and_others":
            fns.discard(EXP)
            fns.discard(LN)


def _build_nc(iters: int):
    _patch_act_tables()
    nc = bacc.Bacc("TRN2", target_bir_lowering=False)

    xT_d = nc.dram_tensor("xT", [C, 4 * SLN], F32, kind="ExternalInput")
    cosT_d = nc.dram_tensor("cosT", [128, 4 * SLN], F32, kind="ExternalInput")
    sinswT_d = nc.dram_tensor("sinswT", [128, 4 * SLN], F32, kind="ExternalInput")
    wq_d = nc.dram_tensor("wqT", [C, C], F32, kind="ExternalInput")
    wk_d = nc.dram_tensor("wkT", [C, C], F32, kind="ExternalInput")
    wv_d = nc.dram_tensor("wvT", [C, C], F32, kind="ExternalInput")
    wo_d = nc.dram_tensor("woT", [C, C], F32, kind="ExternalInput")
    pt_d = nc.dram_tensor("pT", [128, 128], F32, kind="ExternalInput")
    bo_d = nc.dram_tensor("bo", [1, C], F32, kind="ExternalInput")
    out_d = nc.dram_tensor("out", [2 * SLN, C], F32, kind="ExternalOutput")

    def whalf_ap(w_d, half):
        # DRAM [C, C] viewed as SBUF [128, 4, C]: partition = row % 128 within
        # the half, middle dim = which 128-row block of the half.
        return bass.AP(
            tensor=w_d.ap().tensor,
            offset=half * 4 * 128 * C,
            ap=[[C, 128], [128 * C, 4], [1, C]],
        )

    with tile.TileContext(nc) as tc:
        import contextlib

        ctx = contextlib.ExitStack()
        with ctx:
            const_p = ctx.enter_context(tc.tile_pool(name="const", bufs=1))
            w_p = ctx.enter_context(tc.tile_pool(name="wp", bufs=3))
            x_p = ctx.enter_context(tc.tile_pool(name="xp", bufs=2))
            freq_p = ctx.enter_context(tc.tile_pool(name="freq", bufs=1))
            kt_p = ctx.enter_context(tc.tile_pool(name="ktp", bufs=1))
            v_p = ctx.enter_context(tc.tile_pool(name="vp", bufs=1))
            qt_p = ctx.enter_context(tc.tile_pool(name="qtp", bufs=1))
            ot_p = ctx.enter_context(tc.tile_pool(name="otp", bufs=1))
            at_p = ctx.enter_context(tc.tile_pool(name="atp", bufs=1))
            st_p = ctx.enter_context(tc.tile_pool(name="stage", bufs=1))
            outst_p = ctx.enter_context(tc.tile_pool(name="outst", bufs=2))

            # Constants
            pT = const_p.tile([128, 128], F32R, name="pT")
            nc.gpsimd.dma_start(out=pT, in_=pt_d[:, :])
            boB = const_p.tile([128, C], F32, name="boB")
            nc.gpsimd.dma_start(
                out=boB,
                in_=bass.AP(tensor=bo_d.ap().tensor, offset=0, ap=[[0, 128], [1, C]]),
            )
            ones_f = const_p.tile([1, 64], F32, name="ones_f")
            nc.vector.memset(ones_f, 1.0)
            ones64 = const_p.tile([1, 64], F32R, name="ones64")
            nc.vector.tensor_copy(ones64, ones_f)

            def proj_rope(ps_pool, f, wtiles, dest, nch, tok0, cosS, sinS, pfx):
                """Project tokens [tok0, tok0+nch*CH) and apply RoPE into
                dest[hp][:, (within-dest col base)...]. dest cols follow the
                chunk order starting at 0."""
                for ch in range(nch):
                    xs = x_p.tile([128, NCI, CH], F32R, name=f"x{pfx}{f}_{ch}", tag="xs")
                    for ci in range(NCI):
                        nc.gpsimd.dma_start(
                            out=xs[:, ci, :],
                            in_=xT_d[ci * 128:(ci + 1) * 128,
                                     tok0 + ch * CH: tok0 + (ch + 1) * CH])
                    for hp in range(NHP):
                        kp = ps_pool.tile([128, CH], F32, name=f"p{pfx}{f}_{ch}_{hp}", tag="proj")
                        for ci in range(NCI):
                            nc.tensor.matmul(
                                kp, wtiles[ci // 4][:, ci % 4, hp * 128:(hp + 1) * 128],
                                xs[:, ci, :], start=(ci == 0), stop=(ci == NCI - 1))
                        m = st_p.tile([128, CH], F32R, name=f"m{pfx}{f}_{ch}_{hp}", tag="m")
                        nc.vector.tensor_mul(m, kp, sinS[:, ch * CH:(ch + 1) * CH])
                        r = ps_pool.tile([128, CH], F32, name=f"r{pfx}{f}_{ch}_{hp}", tag="rot")
                        nc.tensor.matmul(r, pT, m, start=True, stop=True)
                        c_t = st_p.tile([128, CH], F32, name=f"c{pfx}{f}_{ch}_{hp}", tag="c")
                        nc.vector.tensor_mul(c_t, kp, cosS[:, ch * CH:(ch + 1) * CH])
                        nc.vector.tensor_add(dest[hp][:, ch * CH:(ch + 1) * CH], c_t, r)

            def body():
                for f in range(2):
                    kv0 = f * NKV            # kv token base in slot space
                    q0 = 1152 - 576 * f      # own-frame token base in slot space

                    # ================= projections =================
                    with tc.tile_pool(name=f"psp{f}", bufs=2, space="PSUM") as psp:
                        wk = [w_p.tile([128, 4, C], F32R, name=f"wk{f}_{h}", tag="w")
                              for h in range(2)]
                        for h in range(2):
                            nc.gpsimd.dma_start(out=wk[h], in_=whalf_ap(wk_d, h))
                        cosK = freq_p.tile([128, NKV], F32, name=f"cosK{f}", tag="cosS")
                        sinK = freq_p.tile([128, NKV], F32, name=f"sinK{f}", tag="sinS")
                        nc.sync.dma_start(out=cosK, in_=cosT_d[:, kv0:kv0 + NKV])
                        nc.sync.dma_start(out=sinK, in_=sinswT_d[:, kv0:kv0 + NKV])

                        K_T = [kt_p.tile([128, NKV], F32R, name=f"KT{f}_{hp}", tag=f"KT{hp}")
                               for hp in range(NHP)]
                        proj_rope(psp, f, wk, K_T, NKV // CH, kv0, cosK, sinK, "k")

                        # ---- V projection ----
                        wv = [w_p.tile([128, 4, C], F32R, name=f"wv{f}_{h}", tag="w")
                              for h in range(2)]
                        for h in range(2):
                            nc.gpsimd.dma_start(out=wv[h], in_=whalf_ap(wv_d, h))
                        V = [v_p.tile([128, 16 * 65], BF16, name=f"V{f}_{tt}", tag=f"V{tt}")
                             for tt in range(KK)]
                        for tt in range(KK):
                            v3 = V[tt].rearrange("p (g e) -> p g e", e=65)
                            nc.vector.memset(v3[:, :, 64:65], 1.0)
                            xv = x_p.tile([128, NCI, 128], F32R, name=f"xv{f}_{tt}", tag="xs")
                            for ci in range(NCI):
                                nc.gpsimd.dma_start(
                                    out=xv[:, ci, :],
                                    in_=xT_d[ci * 128:(ci + 1) * 128,
                                             kv0 + tt * 128: kv0 + (tt + 1) * 128])
                            for co in range(2):
                                vp = psp.tile([128, 512], F32, name=f"vp{f}_{tt}_{co}", tag="vproj")
                                for ci in range(NCI):
                                    nc.tensor.matmul(
                                        vp, xv[:, ci, :],
                                        wv[ci // 4][:, ci % 4, co * 512:(co + 1) * 512],
                                        start=(ci == 0), stop=(ci == NCI - 1))
                                nc.vector.tensor_copy(
                                    v3[:, co * 8:(co + 1) * 8, 0:64],
                                    vp.rearrange("p (h d) -> p h d", d=64))

                        # ---- Q projection + rope ----
                        wq = [w_p.tile([128, 4, C], F32R, name=f"wq{f}_{h}", tag="w")
                              for h in range(2)]
                        for h in range(2):
                            nc.gpsimd.dma_start(out=wq[h], in_=whalf_ap(wq_d, h))
                        cosQ = freq_p.tile([128, SLN], F32, name=f"cosQ{f}", tag="cosS")
                        sinQ = freq_p.tile([128, SLN], F32, name=f"sinQ{f}", tag="sinS")
                        nc.sync.dma_start(out=cosQ, in_=cosT_d[:, q0:q0 + SLN])
                        nc.sync.dma_start(out=sinQ, in_=sinswT_d[:, q0:q0 + SLN])
                        Q_T = [qt_p.tile([128, SLN], F32R, name=f"QT{f}_{hp}", tag=f"QT{hp}")
                               for hp in range(NHP)]
                        proj_rope(psp, f, wq, Q_T, SLN // CH, q0, cosQ, sinQ, "q")

                    # ================= attention =================
                    O_T = [ot_p.tile([128, SLN], F32R, name=f"OT{f}_{hp}", tag=f"OT{hp}")
                           for hp in range(NHP)]
                    with tc.tile_pool(name=f"psa{f}", bufs=1, space="PSUM") as psa:
                        S = psa.tile([128, 6, CH], F32, name=f"S{f}", tag="S",
                                     padded_shape=[128, 6, 512])
                        Op = psa.tile([128, 2, CH], F32, name=f"Opp{f}", tag="Op",
                                      padded_shape=[128, 2, 512])
                        for hp in range(NHP):
                            for hh in range(2):
                                hs = slice(hh * 64, hh * 64 + 64)
                                g = 2 * hp + hh  # global head
                                aT = at_p.tile([128, KK, SLN], BF16,
                                               name=f"aT{f}_{hp}_{hh}", tag="aT")
                                aTf = aT.rearrange("p kk n -> p (kk n)")
                                for l in range(18):   # local score tile index
                                    kk, cq = divmod(l, 2)
                                    slot = l % 6
                                    nc.tensor.matmul(
                                        S[:, slot, :],
                                        K_T[hp][hs, kk * 128:(kk + 1) * 128],
                                        Q_T[hp][hs, cq * CH:(cq + 1) * CH],
                                        start=True, stop=True)
                                    if l % 3 == 2:   # exp groups of 3 slots
                                        s0 = slot - 2
                                        nc.scalar.activation(
                                            aTf[:, (l - 2) * CH:(l + 1) * CH],
                                            S[:, s0:s0 + 3, :], EXP, scale=0.125)
                                for cq in range(2):
                                    for kk in range(KK):
                                        nc.tensor.matmul(
                                            Op[0:65, cq, :],
                                            V[kk][:, g * 65:(g + 1) * 65],
                                            aT[:, kk, cq * CH:(cq + 1) * CH],
                                            start=(kk == 0), stop=(kk == KK - 1))
                                ln_row = st_p.tile([1, 2, CH], F32R,
                                                   name=f"ln{f}_{g}", tag="ln")
                                recipB = st_p.tile([64, 2, CH], F32,
                                                   name=f"rb{f}_{g}", tag="rb")
                                for cq in range(2):
                                    nc.scalar.activation(ln_row[:, cq, :],
                                                         Op[64:65, cq, :], LN)
                                    nc.tensor.matmul(S[0:64, 2 + cq, :], ones64,
                                                     ln_row[:, cq, :],
                                                     start=True, stop=True)
                                    nc.scalar.activation(recipB[:, cq, :],
                                                         S[0:64, 2 + cq, :], EXP,
                                                         scale=-1.0)
                                    nc.vector.tensor_mul(
                                        O_T[hp][hs, cq * CH:(cq + 1) * CH],
                                        Op[0:64, cq, :], recipB[:, cq, :])

                    # ================= output projection =================
                    with tc.tile_pool(name=f"pso{f}", bufs=2, space="PSUM") as pso:
                        wo = [w_p.tile([128, 4, C], F32R, name=f"wo{f}_{h}", tag="w")
                              for h in range(2)]
                        for h in range(2):
                            nc.gpsimd.dma_start(out=wo[h], in_=whalf_ap(wo_d, h))
                        for t0, tl in [(0, 128), (128, 128), (256, 128), (384, 128), (512, 64)]:
                            outst = outst_p.tile([128, C], F32, name=f"os{f}_{t0}", tag="os")
                            for co in range(2):
                                op = pso.tile([128, 512], F32,
                                              name=f"op{f}_{t0}_{co}", tag="oproj")
                                for hp in range(NHP):
                                    nc.tensor.matmul(
                                        op[0:tl, :], O_T[hp][:, t0:t0 + tl],
                                        wo[hp // 4][:, hp % 4, co * 512:(co + 1) * 512],
                                        start=(hp == 0), stop=(hp == NHP - 1))
                                nc.vector.tensor_add(
                                    outst[0:tl, co * 512:(co + 1) * 512],
                                    op[0:tl, :], boB[0:tl, co * 512:(co + 1) * 512])
                            nc.sync.dma_start(
                                out=out_d[f * SLN + t0: f * SLN + t0 + tl, :],
                                in_=outst[0:tl, :])

            if iters > 1:
                with tc.For_i(0, iters, 1):
                    body()
            else:
                body()

    nc.compile()
    return nc


_nc_cache = {}


def _get_nc(iters=1):
    if iters not in _nc_cache:
        _nc_cache[iters] = _build_nc(iters)
    return _nc_cache[iters]


def _host_prep(img, freqs_cos, freqs_sin, Wq, Wk, Wv, Wo, bo):
    img = np.asarray(img, dtype=np.float32)
    freqs_cos = np.asarray(freqs_cos, dtype=np.float32)
    freqs_sin = np.asarray(freqs_sin, dtype=np.float32)

    # pair-swapped sin: sinsw[2i] = sin[2i+1], sinsw[2i+1] = sin[2i]
    sw = np.arange(HD).reshape(-1, 2)[:, ::-1].reshape(-1)
    sinsw = freqs_sin[:, sw]

    cos_f = freqs_cos.reshape(T, N, HD)
    sin_f = sinsw.reshape(T, N, HD)

    # rot(x)[2i] = -x[2i+1], rot(x)[2i+1] = x[2i]; as rot = P @ x (per 64-dim
    # head half, tiled to 128); the matmul takes P^T as the stationary side.
    P = np.zeros((128, 128), np.float32)
    for i in range(64):
        P[2 * i, 2 * i + 1] = -1.0
        P[2 * i + 1, 2 * i] = 1.0
    pT = np.ascontiguousarray(P.T)

    wqT = np.ascontiguousarray(np.asarray(Wq, np.float32).T)
    wkT = np.ascontiguousarray(np.asarray(Wk, np.float32).T)
    wvT = np.ascontiguousarray(np.asarray(Wv, np.float32).T)
    woT = np.ascontiguousarray(np.asarray(Wo, np.float32).T)
    bo2 = np.asarray(bo, dtype=np.float32).reshape(1, C)

    in_maps = []
    for core in range(NCORES):
        b, fp = divmod(core, 4)
        own0, own1 = 2 * fp, 2 * fp + 1
        lo = own0 - 1 if fp > 0 else 1
        hi = own1 + 1 if fp < 3 else 6
        slots = [lo, own1, own0, hi]
        xT = np.ascontiguousarray(img[b, slots].reshape(4 * N, C).T)
        cosT = np.ascontiguousarray(cos_f[slots].reshape(4 * N, HD).T)
        sinT = np.ascontiguousarray(sin_f[slots].reshape(4 * N, HD).T)
        in_maps.append({
            "xT": xT,
            "cosT": np.concatenate([cosT, cosT], axis=0),
            "sinswT": np.concatenate([sinT, sinT], axis=0),
            "wqT": wqT, "wkT": wkT, "wvT": wvT, "woT": woT,
            "pT": pT, "bo": bo2,
        })
    return in_maps


def kernel(img, freqs_cos, freqs_sin, Wq, Wk, Wv, Wo, bo, _iters=1):
    in_maps = _host_prep(img, freqs_cos, freqs_sin, Wq, Wk, Wv, Wo, bo)
    nc = _get_nc(_iters)
    res = run_bass_kernel_spmd(nc, in_maps, core_ids=list(range(NCORES)))
    out = np.zeros((B, T, N, C), np.float32)
    for core in range(NCORES):
        b, fp = divmod(core, 4)
        r = res.results[core]["out"].reshape(2, N, C)
        out[b, 2 * fp] = r[0]
        out[b, 2 * fp + 1] = r[1]
    return out
